# revision 1
# baseline (speedup 1.0000x reference)
"""Causal self-attention (T=2048, D=2048, H=16) on 8 Trainium2 NeuronCores.

Head-sharded tensor parallel: 2 heads per core. Each core computes its
heads' q/k/v projections, causal attention, then the cores AllGather the
attention outputs (feature-major) and each computes a 256-column slice of
the output projection. The host concatenates the slices.

Layouts (all feature/d-major so the PE contracts along partitions):
  - xT      [D, T]    : x transposed (host-side)
  - wqkvT   [D, 768]  : this core's W_attn rows (q0 q1 k0 k1 v0 v1), transposed
  - wpT     [D, 256]  : this core's W_proj rows, transposed
  - qT/kT   [128, T] per head (feature on partition)
  - v       [tok, 256] (token on partition) so P.T@V needs no transpose
  - S_T     [tk, tq] so softmax sums run via an all-ones matmul on the PE
Matmul inputs are float32r (TF32-like) for 4x PE throughput vs fp32.
"""

import numpy as np

import concourse.bacc as bacc
import concourse.bass_utils as bass_utils
import concourse.mybir as mybir
import concourse.tile as tile

T = 2048
D = 2048
H = 16
C = 128
N_CORES = 8
HPC = H // N_CORES          # heads per core = 2
FPC = HPC * C               # features per core = 256
TQB = 512                   # tq block (PSUM free-dim limit for fp32)
NTQ = T // TQB              # 4
NKT = T // 128              # 16 tk tiles
ND = D // 128               # 16 contraction tiles
SCALE = 1.0 / np.sqrt(np.float32(C))

FR = mybir.dt.float32r
F32 = mybir.dt.float32

_NC_CACHE = {}


def build_nc(sim_single_core=False, reps=1, phases=3, ag_chunks=4):
    key = ("sim" if sim_single_core else "nc") + f"_{reps}_{phases}_{ag_chunks}"
    if key in _NC_CACHE:
        return _NC_CACHE[key]
    ndev = 1 if sim_single_core else N_CORES
    nc = bacc.Bacc("TRN2", target_bir_lowering=False, debug=False, num_devices=ndev)

    xT = nc.dram_tensor("xT", [D, T], FR, kind="ExternalInput").ap()
    wqkvT = nc.dram_tensor("wqkvT", [D, 3 * FPC], FR, kind="ExternalInput").ap()
    wpT = nc.dram_tensor("wpT", [D, FPC], FR, kind="ExternalInput").ap()
    # mask band: maskB[p, j] = 1.0 if p <= j - 384 else 0.0  (j in [0, 896))
    maskB = nc.dram_tensor("maskB", [128, 896], F32, kind="ExternalInput").ap()
    if phases == 3:
        yT = nc.dram_tensor("yT", [FPC, T], F32, kind="ExternalOutput").ap()
    elif phases == 2:
        yT = nc.dram_tensor("yT", [FPC, T], FR, kind="ExternalOutput").ap()
    else:
        yT = nc.dram_tensor("yT", [128, 6 * T], FR, kind="ExternalOutput").ap()

    if ag_chunks == 1:
        ag_in_full = nc.dram_tensor("ag_in", [FPC, T], FR, kind="Internal").ap()
        ag_sh_full = nc.dram_tensor("ag_sh", [D, T], FR, kind="Internal",
                                    addr_space="Shared").ap()
        ag_in = [ag_in_full[:, j * TQB:(j + 1) * TQB] for j in range(NTQ)]
        ag_sh = [ag_sh_full[:, j * TQB:(j + 1) * TQB] for j in range(NTQ)]
        ag_full = (ag_in_full, ag_sh_full)
    else:
        ag_in = [nc.dram_tensor(f"ag_in{j}", [FPC, TQB], FR, kind="Internal").ap()
                 for j in range(NTQ)]
        ag_sh = [nc.dram_tensor(f"ag_sh{j}", [D, TQB], FR, kind="Internal",
                                addr_space="Shared").ap()
                 for j in range(NTQ)]
        ag_full = None

    with tile.TileContext(nc) as tc:
        with tc.tile_pool(name="persist", bufs=1) as pp, \
             tc.tile_pool(name="ptiles", bufs=8) as ppt, \
             tc.tile_pool(name="small", bufs=2) as smp, \
             tc.tile_pool(name="psA", bufs=4, space="PSUM") as psA, \
             tc.tile_pool(name="psB", bufs=2, space="PSUM") as psB, \
             tc.tile_pool(name="psC", bufs=2, space="PSUM") as psC:

            mask_sb = pp.tile([128, 896], F32, tag="mask")
            ones_f = pp.tile([128, 128], F32, tag="onesf")
            ones_r = pp.tile([128, 128], FR, tag="onesr")
            nc.vector.memset(ones_f[:], 1.0)
            nc.vector.tensor_copy(ones_r[:], ones_f[:])
            nc.sync.dma_start(mask_sb[:], maskB[:])

            for _rep in range(reps):
                emit_body(nc, tc, pp, ppt, smp, psA, psB, psC,
                          xT, wqkvT, wpT, yT, ag_in, ag_sh,
                          mask_sb, ones_r, sim_single_core, phases, ag_full)

    nc.compile()
    _NC_CACHE[key] = nc
    return nc


def emit_body(nc, tc, pp, ppt, smp, psA, psB, psC,
              xT, wqkvT, wpT, yT, ag_in, ag_sh, mask_sb, ones_r,
              sim_single_core, phases=3, ag_full=None):
    # ---- phase 1: QKV projections ----
    # qkT layout: feature-block fb in {q_h0, q_h1, k_h0, k_h1} at cols
    # [fb*T, (fb+1)*T); v_sb: tok-tile tt at cols [tt*FPC, ...).
    qkT = pp.tile([128, 4 * T], FR, tag="qkT")               # 32KB/part
    v_sb = pp.tile([128, NKT * FPC], FR, tag="v")            # 16KB/part
    ph1_cm = tc.tile_pool(name="ph1", bufs=1)
    sp_cm = tc.tile_pool(name="stream", bufs=2)
    ph1 = ph1_cm.__enter__()
    sp = sp_cm.__enter__()
    w_sb = ph1.tile([128, ND * 3 * FPC], FR, tag="wbig")     # 48KB/part
    xcols = []
    for tb in range(NTQ):
        xcols.append(sp.tile([128, ND * TQB], FR, tag="xcol", name=f"xcol{tb}"))  # 32KB/part
    # interleave DMA emission so the first-needed tiles land first
    for t in range(ND):
        nc.sync.dma_start(
            w_sb[:, t * 3 * FPC:(t + 1) * 3 * FPC],
            wqkvT[t * 128:(t + 1) * 128, :])
        nc.sync.dma_start(
            xcols[0][:, t * TQB:(t + 1) * TQB],
            xT[t * 128:(t + 1) * 128, 0:TQB])
    for tb in range(NTQ):
        xcol = xcols[tb]
        if tb > 0:
            for t in range(ND):
                nc.sync.dma_start(
                    xcol[:, t * TQB:(t + 1) * TQB],
                    xT[t * 128:(t + 1) * 128, tb * TQB:(tb + 1) * TQB])
        for fb in range(4):
            ps = psA.tile([128, TQB], F32, tag="a")
            for t in range(ND):
                nc.tensor.matmul(
                    ps[:],
                    w_sb[:, t * 3 * FPC + fb * 128: t * 3 * FPC + fb * 128 + 128],
                    xcol[:, t * TQB:(t + 1) * TQB],
                    start=(t == 0), stop=(t == ND - 1))
            nc.vector.tensor_copy(
                qkT[:, fb * T + tb * TQB: fb * T + (tb + 1) * TQB], ps[:])
        for tt in range(4):
            tok = tb * 4 + tt
            ps = psB.tile([128, FPC], F32, tag="b")
            for t in range(ND):
                nc.tensor.matmul(
                    ps[:],
                    xcol[:, t * TQB + tt * 128: t * TQB + (tt + 1) * 128],
                    w_sb[:, t * 3 * FPC + 2 * FPC:(t + 1) * 3 * FPC],
                    start=(t == 0), stop=(t == ND - 1))
            nc.vector.tensor_copy(v_sb[:, tok * FPC:(tok + 1) * FPC], ps[:])

    sp_cm.__exit__(None, None, None)
    ph1_cm.__exit__(None, None, None)

    if phases == 1:
        nc.sync.dma_start(yT[:, 0:4 * T], qkT[:])
        nc.sync.dma_start(yT[:, 4 * T:4 * T + NKT * FPC], v_sb[:])
        return

    # ---- phase 2: causal attention, software-pipelined ----
    # The PE executes its queue in order, so S(tk+LOOKAHEAD) is emitted
    # BEFORE sum/PV(tk): while exp(tk) runs on ACT, the PE computes future
    # S tiles instead of stalling on the ACT/DVE round-trip.
    # AllGather + projection for tq block j fire as soon as both heads of
    # block j finish, overlapping the collective + proj DMA with attention
    # of later blocks.
    wpp_cm = tc.tile_pool(name="wproj", bufs=1)
    agp_cm = tc.tile_pool(name="agpool", bufs=16)
    wpp = wpp_cm.__enter__()
    agp = agp_cm.__enter__()
    wp_sb = wpp.tile([128, ND * FPC], FR, tag="wp")
    for t in range(ND):
        nc.sync.dma_start(
            wp_sb[:, t * FPC:(t + 1) * FPC], wpT[t * 128:(t + 1) * 128, :])
    LOOKAHEAD = 1
    for j in range(NTQ):
        n_tk = 4 * (j + 1)                  # causal: tk tiles 0..4j+3
        sum_ps = {}
        o_ps = {}
        for h in range(HPC):
            sum_ps[h] = psB.tile([128, TQB], F32, tag="b", name=f"sum{j}{h}")
            o_ps[h] = psC.tile([128, TQB], F32, tag="c", name=f"ops{j}{h}")
        p_tiles = {}

        def emit_s_exp(h, tk, j=j):
            qh = qkT[:, h * T:(h + 1) * T]
            kh = qkT[:, (2 + h) * T:(3 + h) * T]
            s_ps = psA.tile([128, TQB], F32, tag="a", name=f"s{j}{h}{tk}")
            nc.tensor.matmul(
                s_ps[:],
                kh[:, tk * 128:(tk + 1) * 128],
                qh[:, j * TQB:(j + 1) * TQB],
                start=True, stop=True)
            p_sb = ppt.tile([128, TQB], FR, tag="p", name=f"p{j}{h}{tk}")
            nc.scalar.activation(
                p_sb[:], s_ps[:], mybir.ActivationFunctionType.Exp,
                scale=float(SCALE))
            delta = tk * 128 - j * TQB
            if delta >= 0:                  # diagonal tile: causal mask
                nc.vector.tensor_mul(
                    p_sb[:], p_sb[:], mask_sb[:, 384 - delta: 896 - delta])
            return p_sb

        for tk in range(min(LOOKAHEAD, n_tk)):
            for h in range(HPC):
                p_tiles[h, tk] = emit_s_exp(h, tk)
        for tk in range(n_tk):
            if tk + LOOKAHEAD < n_tk:
                for h in range(HPC):
                    p_tiles[h, tk + LOOKAHEAD] = emit_s_exp(h, tk + LOOKAHEAD)
            for h in range(HPC):
                p_sb = p_tiles.pop((h, tk))
                nc.tensor.matmul(
                    sum_ps[h][:], ones_r[:], p_sb[:],
                    start=(tk == 0), stop=(tk == n_tk - 1))
                nc.tensor.matmul(
                    o_ps[h][:],
                    v_sb[:, tk * FPC + h * 128: tk * FPC + (h + 1) * 128],
                    p_sb[:],
                    start=(tk == 0), stop=(tk == n_tk - 1))
        for h in range(HPC):
            inv_sb = smp.tile([128, TQB], F32, tag="inv", name=f"inv{j}{h}")
            nc.vector.reciprocal(inv_sb[:], sum_ps[h][:])
            o_sb = smp.tile([128, TQB], FR, tag="osb", name=f"osb{j}{h}")
            nc.vector.tensor_mul(o_sb[:], o_ps[h][:], inv_sb[:])
            if phases == 2:
                nc.sync.dma_start(
                    yT[h * 128:(h + 1) * 128, j * TQB:(j + 1) * TQB], o_sb[:])
            else:
                nc.sync.dma_start(ag_in[j][h * 128:(h + 1) * 128, :], o_sb[:])

        # ---- phase 3+4 for block j: AllGather + projection slice ----
        if phases == 2:
            continue
        if ag_full is not None:
            continue
        if sim_single_core:
            nc.sync.dma_start(ag_sh[j][0:FPC, :], ag_in[j][:])
        else:
            nc.gpsimd.collective_compute(
                "AllGather", mybir.AluOpType.bypass,
                replica_groups=[list(range(N_CORES))],
                ins=[ag_in[j][:]], outs=[ag_sh[j][:]])
        # lag the projection two blocks behind the collective so the
        # AG + 4MB proj DMA complete under later attention blocks
        if j >= 2:
            emit_proj_block(nc, wp_sb, agp, smp, psA, psC, yT, ag_sh, j - 2)
    if phases == 3 and ag_full is None:
        for j in (NTQ - 2, NTQ - 1):
            emit_proj_block(nc, wp_sb, agp, smp, psA, psC, yT, ag_sh, j)
    if phases == 3 and ag_full is not None:
        ag_in_full, ag_sh_full = ag_full
        if sim_single_core:
            nc.sync.dma_start(ag_sh_full[0:FPC, :], ag_in_full[:])
        else:
            nc.gpsimd.collective_compute(
                "AllGather", mybir.AluOpType.bypass,
                replica_groups=[list(range(N_CORES))],
                ins=[ag_in_full[:]], outs=[ag_sh_full[:]])
        for j in range(NTQ):
            emit_proj_block(nc, wp_sb, agp, smp, psA, psC, yT, ag_sh, j)

    agp_cm.__exit__(None, None, None)
    wpp_cm.__exit__(None, None, None)


def emit_proj_block(nc, wp_sb, agp, smp, psA, psC, yT, ag_sh, j):
    ps0 = psA.tile([128, TQB], F32, tag="a")
    ps1 = psC.tile([128, TQB], F32, tag="c")
    for t in range(ND):
        agt = agp.tile([128, TQB], FR, tag="agt")
        nc.sync.dma_start(agt[:], ag_sh[j][t * 128:(t + 1) * 128, :])
        nc.tensor.matmul(
            ps0[:], wp_sb[:, t * FPC: t * FPC + 128], agt[:],
            start=(t == 0), stop=(t == ND - 1))
        nc.tensor.matmul(
            ps1[:], wp_sb[:, t * FPC + 128: t * FPC + 256], agt[:],
            start=(t == 0), stop=(t == ND - 1))
    for oc, ps in ((0, ps0), (1, ps1)):
        y_sb = smp.tile([128, TQB], F32, tag="ysb")
        nc.vector.tensor_copy(y_sb[:], ps[:])
        nc.sync.dma_start(
            yT[oc * 128:(oc + 1) * 128, j * TQB:(j + 1) * TQB], y_sb[:])


def make_mask_band() -> np.ndarray:
    p = np.arange(128)[:, None]
    j = np.arange(896)[None, :]
    return (p <= j - 384).astype(np.float32)


def prepare_in_maps(x, W_attn, W_proj):
    x = np.ascontiguousarray(np.asarray(x, dtype=np.float32))
    W_attn = np.ascontiguousarray(np.asarray(W_attn, dtype=np.float32))
    W_proj = np.ascontiguousarray(np.asarray(W_proj, dtype=np.float32))
    xT = np.ascontiguousarray(x.T)
    mask = make_mask_band()
    in_maps = []
    for r in range(N_CORES):
        rows = slice(r * FPC, (r + 1) * FPC)
        w_qkv = np.concatenate(
            [W_attn[0 * D:][rows], W_attn[1 * D:][rows], W_attn[2 * D:][rows]],
            axis=0)                                   # [768, D]
        in_maps.append({
            "xT": xT,
            "wqkvT": np.ascontiguousarray(w_qkv.T),   # [D, 768]
            "wpT": np.ascontiguousarray(W_proj[rows].T),  # [D, 256]
            "maskB": mask,
        })
    return in_maps


def postprocess(results) -> np.ndarray:
    return np.concatenate([r["yT"].T for r in results], axis=1)


def kernel(x, W_attn, W_proj) -> np.ndarray:
    nc = build_nc()
    in_maps = prepare_in_maps(x, W_attn, W_proj)
    res = bass_utils.run_bass_kernel_spmd(
        nc, in_maps, core_ids=list(range(N_CORES)), trace=False)
    return postprocess(res.results)



# revision 11
# speedup vs baseline: 2.7966x; 2.7966x over previous
"""Causal self-attention (T=2048, D=2048, H=16) on 8 Trainium2 NeuronCores.

Head-sharded tensor parallel attention (2 heads/core) + token-sharded
output projection, exchanged via AllToAll:

  phase 1: each core computes q/k/v for its 2 heads over all T tokens.
  phase 2: causal attention. S^T = K^T Q is computed [tk, tq]; exp on ACT;
    PV runs TRANSPOSED: stationary = 128-token P chunk, moving = V tile
    [tk, 256 feats both heads] -> o accumulates TOKEN-major [tok, feat].
    The softmax denominator rides along as a 1-column matmul on the same
    stationary P chunk (ones moving vector) - near-zero PE cost.
    Causal trimming: fully-masked chunks of diagonal tiles are skipped.
  phase 3: token-major o goes through an AllToAll (two halves, pipelined
    under attention): core i ends with ALL features of o for its tokens.
    Wire traffic per core is (N-1)/N * 1MB vs 7.35MB for the old
    AllGather - the AG chain was the old kernel's critical path.
  phase 4: each core computes yT[:, its tokens] = Wp @ oT with the full
    W_proj.T held in SBUF (persists across reps), after PE-transposing
    the received token-major tiles back to feature-major.

All matmul operands are bf16 (fp32 PSUM accumulation): same PE rate as
fp32r but half the DMA/SBUF traffic; rel err ~3e-3 vs the 2e-2 gate.
"""

import numpy as np

import concourse.bacc as bacc
import concourse.bass_utils as bass_utils
import concourse.mybir as mybir
import concourse.tile as tile

T = 2048
D = 2048
H = 16
C = 128
N_CORES = 8
HPC = H // N_CORES          # heads per core = 2
FPC = HPC * C               # features per core = 256
TQB = 512                   # tq block (PSUM free-dim limit for fp32)
NTQ = T // TQB              # 4
NKT = T // 128              # 16 tk tiles
ND = D // 128               # 16 contraction tiles
THALF = T // 2              # a2a half size (tokens)
TPC = T // N_CORES          # tokens per core after a2a = 256
SCALE = 1.0 / np.sqrt(np.float32(C))

FR = mybir.dt.float32r
F32 = mybir.dt.float32
DT = mybir.dt.bfloat16

_NC_CACHE = {}


def build_nc(sim_single_core=False, reps=1, phases=3, lookahead=2):
    key = ("sim" if sim_single_core else "nc") + f"_{reps}_{phases}_{lookahead}"
    if key in _NC_CACHE:
        return _NC_CACHE[key]
    ndev = 1 if sim_single_core else N_CORES
    nc = bacc.Bacc("TRN2", target_bir_lowering=False, debug=False, num_devices=ndev)

    xT = nc.dram_tensor("xT", [D, T], DT, kind="ExternalInput").ap()
    wqkvT = nc.dram_tensor("wqkvT", [D, 3 * FPC], DT, kind="ExternalInput").ap()
    # full W_proj.T [in_feat, out_feat] - every core holds all of it
    wpT = nc.dram_tensor("wpT", [D, D], DT, kind="ExternalInput").ap()
    # mask band: maskB[p, j] = 1.0 if p <= j - 384 else 0.0  (j in [0, 896))
    maskB = nc.dram_tensor("maskB", [128, 896], F32, kind="ExternalInput").ap()
    eyeB = nc.dram_tensor("eyeB", [128, 128], DT, kind="ExternalInput").ap()
    if phases == 3:
        # all 2048 out-features x my 256 tokens (2 halves of 128)
        yT = nc.dram_tensor("yT", [D, TPC], DT, kind="ExternalOutput").ap()
    elif phases == 2:
        yT = nc.dram_tensor("yT", [T, FPC], DT, kind="ExternalOutput").ap()
    else:
        yT = nc.dram_tensor("yT", [128, 6 * T], DT, kind="ExternalOutput").ap()

    a2a_in = nc.dram_tensor("a2a_in", [T, FPC], DT, kind="Internal").ap()
    a2a_out = [nc.dram_tensor(f"a2a_out{hf}", [THALF, FPC], DT,
                              kind="Internal").ap()
               for hf in range(2)]

    with tile.TileContext(nc) as tc:
        with tc.tile_pool(name="persist", bufs=1) as pp, \
             tc.tile_pool(name="ptiles", bufs=8) as ppt, \
             tc.tile_pool(name="small", bufs=4) as smp, \
             tc.tile_pool(name="a2ald", bufs=6) as a2ap, \
             tc.tile_pool(name="otp", bufs=1) as otp, \
             tc.tile_pool(name="psA", bufs=3, space="PSUM") as psA, \
             tc.tile_pool(name="psO", bufs=2, space="PSUM") as psO, \
             tc.tile_pool(name="psS", bufs=1, space="PSUM") as psS, \
             tc.tile_pool(name="psP", bufs=1, space="PSUM") as psP, \
             tc.tile_pool(name="psT", bufs=1, space="PSUM") as psT:

            mask_sb = pp.tile([128, 896], F32, tag="mask")
            ones_sb = pp.tile([128, 128], DT, tag="ones")
            eye_sb = pp.tile([128, 128], DT, tag="eye")
            nc.vector.memset(ones_sb[:], 1.0)
            nc.sync.dma_start(mask_sb[:], maskB[:])
            nc.sync.dma_start(eye_sb[:], eyeB[:])
            wp_sb = None
            if phases == 3:
                # [128, t-block * 2048 of-cols]; persists across reps
                wp_sb = pp.tile([128, ND * D], DT, tag="wpfull")  # 64KB/part
                for t in range(ND):
                    nc.sync.dma_start(
                        wp_sb[:, t * D:(t + 1) * D],
                        wpT[t * 128:(t + 1) * 128, :])

            for _rep in range(reps):
                emit_body(nc, tc, pp, ppt, smp, a2ap, otp,
                          psA, psO, psS, psP, psT,
                          xT, wqkvT, yT, a2a_in, a2a_out,
                          mask_sb, ones_sb, eye_sb, wp_sb,
                          sim_single_core, phases, lookahead)

    nc.compile()
    _NC_CACHE[key] = nc
    return nc


def emit_body(nc, tc, pp, ppt, smp, a2ap, otp, psA, psO, psS, psP, psT,
              xT, wqkvT, yT, a2a_in, a2a_out,
              mask_sb, ones_sb, eye_sb, wp_sb,
              sim_single_core, phases=3, lookahead=2):
    # ---- phase 1: QKV projections ----
    # qkT layout: feature-block fb in {q_h0, q_h1, k_h0, k_h1} at cols
    # [fb*T, (fb+1)*T); v_sb: tok-tile tt at cols [tt*FPC, ...).
    qkT = pp.tile([128, 4 * T], DT, tag="qkT")               # 16KB/part
    v_sb = pp.tile([128, NKT * FPC], DT, tag="v")            # 8KB/part
    ph1_cm = tc.tile_pool(name="ph1", bufs=1)
    sp_cm = tc.tile_pool(name="stream", bufs=2)
    ph1 = ph1_cm.__enter__()
    sp = sp_cm.__enter__()
    w_sb = ph1.tile([128, ND * 3 * FPC], DT, tag="wbig")     # 24KB/part
    xcols = []
    for tb in range(NTQ):
        xcols.append(sp.tile([128, ND * TQB], DT, tag="xcol", name=f"xcol{tb}"))
    # interleave DMA emission so the first-needed tiles land first
    for t in range(ND):
        nc.sync.dma_start(
            w_sb[:, t * 3 * FPC:(t + 1) * 3 * FPC],
            wqkvT[t * 128:(t + 1) * 128, :])
        nc.sync.dma_start(
            xcols[0][:, t * TQB:(t + 1) * TQB],
            xT[t * 128:(t + 1) * 128, 0:TQB])
    for tb in range(NTQ):
        xcol = xcols[tb]
        if tb > 0:
            for t in range(ND):
                nc.sync.dma_start(
                    xcol[:, t * TQB:(t + 1) * TQB],
                    xT[t * 128:(t + 1) * 128, tb * TQB:(tb + 1) * TQB])
        for fb in range(4):
            ps = psA.tile([128, TQB], F32, tag="a")
            for t in range(ND):
                nc.tensor.matmul(
                    ps[:],
                    w_sb[:, t * 3 * FPC + fb * 128: t * 3 * FPC + fb * 128 + 128],
                    xcol[:, t * TQB:(t + 1) * TQB],
                    start=(t == 0), stop=(t == ND - 1))
            nc.vector.tensor_copy(
                qkT[:, fb * T + tb * TQB: fb * T + (tb + 1) * TQB], ps[:])
        for tt in range(4):
            tok = tb * 4 + tt
            ps = psO.tile([128, FPC], F32, tag="o")
            for t in range(ND):
                nc.tensor.matmul(
                    ps[:],
                    xcol[:, t * TQB + tt * 128: t * TQB + (tt + 1) * 128],
                    w_sb[:, t * 3 * FPC + 2 * FPC:(t + 1) * 3 * FPC],
                    start=(t == 0), stop=(t == ND - 1))
            nc.scalar.activation(
                v_sb[:, tok * FPC:(tok + 1) * FPC], ps[:],
                mybir.ActivationFunctionType.Copy)

    sp_cm.__exit__(None, None, None)
    ph1_cm.__exit__(None, None, None)

    if phases == 1:
        nc.sync.dma_start(yT[:, 0:4 * T], qkT[:])
        nc.sync.dma_start(yT[:, 4 * T:4 * T + NKT * FPC], v_sb[:])
        return

    # ---- phase 2: causal attention, software-pipelined ----
    # The PE executes its queue in order, so S(tk+LOOKAHEAD) is emitted
    # BEFORE PV(tk): while exp(tk) runs on ACT, the PE computes future
    # S tiles instead of stalling on the ACT round-trip.
    for j in range(NTQ):
        n_tk = 4 * (j + 1)                  # causal: tk tiles 0..4j+3
        sum_ps = psS.tile([128, 16], F32, tag="s", name=f"sum{j}")
        ob = [psO.tile([128, 2 * FPC], F32, tag="o", name=f"ops{j}{b}")
              for b in range(2)]
        o_ps = [ob[c // 2][:, (c % 2) * FPC:(c % 2 + 1) * FPC]
                for c in range(4)]
        p_tiles = {}

        def emit_s_exp(h, tk, j=j):
            qh = qkT[:, h * T:(h + 1) * T]
            kh = qkT[:, (2 + h) * T:(3 + h) * T]
            delta = tk * 128 - j * TQB
            lo = max(delta, 0)
            s_ps = psA.tile([128, TQB], F32, tag="a", name=f"s{j}{h}{tk}")
            nc.tensor.matmul(
                s_ps[:, lo:TQB],
                kh[:, tk * 128:(tk + 1) * 128],
                qh[:, j * TQB + lo:(j + 1) * TQB],
                start=True, stop=True)
            p_sb = ppt.tile([128, TQB], DT, tag="p", name=f"p{j}{h}{tk}")
            nc.scalar.activation(
                p_sb[:, lo:TQB], s_ps[:, lo:TQB],
                mybir.ActivationFunctionType.Exp, scale=float(SCALE))
            if delta >= 0:                  # boundary chunk needs the mask
                nc.vector.tensor_mul(
                    p_sb[:, lo:lo + 128], p_sb[:, lo:lo + 128],
                    mask_sb[:, 384:512])
            return p_sb, lo

        for tk in range(min(lookahead, n_tk)):
            for h in range(HPC):
                p_tiles[h, tk] = emit_s_exp(h, tk)
        for tk in range(n_tk):
            if tk + lookahead < n_tk:
                for h in range(HPC):
                    p_tiles[h, tk + lookahead] = emit_s_exp(h, tk + lookahead)
            for h in range(HPC):
                p_sb, lo = p_tiles.pop((h, tk))
                for c in range(lo // 128, 4):
                    # start=True clears the WHOLE PSUM bank's has_written
                    # bits, so emit it only on the first write into each
                    # bank; per-element bits turn every other group's
                    # first write into an overwrite automatically.
                    last = (tk == 4 * j + c)
                    nc.tensor.matmul(
                        o_ps[c][:, h * 128:(h + 1) * 128],
                        p_sb[:, c * 128:(c + 1) * 128],
                        v_sb[:, tk * FPC + h * 128: tk * FPC + (h + 1) * 128],
                        start=(tk == 0 and h == 0 and c % 2 == 0),
                        stop=last, skip_group_check=True)
                    nc.tensor.matmul(
                        sum_ps[:, 2 * c + h: 2 * c + h + 1],
                        p_sb[:, c * 128:(c + 1) * 128],
                        ones_sb[:, 0:1],
                        start=(tk == 0 and h == 0 and c == 0),
                        stop=last, skip_group_check=True)
            # chunk tk-4j of this block is complete -> normalize + ship
            cd = tk - 4 * j
            if cd >= 0:
                inv_sb = smp.tile([128, 2], F32, tag="inv", name=f"inv{j}{cd}")
                nc.vector.reciprocal(inv_sb[:], sum_ps[:, 2 * cd:2 * cd + 2])
                o_sbT = smp.tile([128, FPC], DT, tag="osb", name=f"osb{j}{cd}")
                for h in range(HPC):
                    nc.vector.tensor_scalar_mul(
                        o_sbT[:, h * 128:(h + 1) * 128],
                        o_ps[cd][:, h * 128:(h + 1) * 128],
                        inv_sb[:, h:h + 1])
                if phases == 2:
                    nc.sync.dma_start(
                        yT[j * TQB + cd * 128: j * TQB + (cd + 1) * 128, :],
                        o_sbT[:])
                else:
                    nc.sync.dma_start(
                        a2a_in[j * TQB + cd * 128: j * TQB + (cd + 1) * 128, :],
                        o_sbT[:])

        if phases == 3 and j % 2 == 1:
            # ---- phase 3: AllToAll for tokens [THALF*(j//2), ...) ----
            hf = j // 2
            if sim_single_core:
                nc.sync.dma_start(
                    a2a_out[hf][:], a2a_in[hf * THALF:(hf + 1) * THALF, :])
            else:
                nc.gpsimd.collective_compute(
                    "AllToAll", mybir.AluOpType.bypass,
                    replica_groups=[list(range(N_CORES))],
                    ins=[a2a_in[hf * THALF:(hf + 1) * THALF, :]],
                    outs=[a2a_out[hf][:]])
            if hf == 0:
                # half-0 projection overlaps attention blocks 2,3
                emit_post_half(nc, a2ap, otp, smp, psP, psT, a2a_out, yT,
                               wp_sb, eye_sb, 0)
    if phases == 3:
        emit_post_half(nc, a2ap, otp, smp, psP, psT, a2a_out, yT,
                       wp_sb, eye_sb, 1)


def emit_post_half(nc, a2ap, otp, smp, psP, psT, a2a_out, yT, wp_sb, eye_sb, hf):
    """Transpose received token-major o tiles to feature-major, then
    yT[:, my 128 tokens of half hf] = Wp @ oT."""
    oT = otp.tile([128, ND * 128], DT, tag="oT", name=f"oT{hf}")  # 4KB/part
    for half8 in range(2):
        ptb = psT.tile([128, 1024], DT, tag="t", name=f"ptb{hf}{half8}")
        for cs4 in range(4):
            cs = half8 * 4 + cs4
            ld = a2ap.tile([128, FPC], DT, tag="ld", name=f"ld{hf}{cs}")
            nc.sync.dma_start(ld[:], a2a_out[hf][cs * 128:(cs + 1) * 128, :])
            for fh in range(2):
                t = cs * 2 + fh
                k = cs4 * 2 + fh
                pst = ptb[:, k * 128:(k + 1) * 128]
                nc.tensor.transpose(pst, ld[:, fh * 128:(fh + 1) * 128],
                                    eye_sb[:])
                nc.scalar.activation(
                    oT[:, t * 128:(t + 1) * 128], pst,
                    mybir.ActivationFunctionType.Copy)
    for ow in range(4):
        ps = psP.tile([128, TQB], F32, tag="pp", name=f"pj{hf}{ow}")
        for t in range(ND):
            for oi in range(4):
                of = ow * 4 + oi
                nc.tensor.matmul(
                    ps[:, oi * 128:(oi + 1) * 128],
                    wp_sb[:, t * D + of * 128: t * D + (of + 1) * 128],
                    oT[:, t * 128:(t + 1) * 128],
                    start=(t == 0 and oi == 0), stop=(t == ND - 1),
                    skip_group_check=True)
        y_sb = smp.tile([128, TQB], DT, tag="ysb", name=f"y{hf}{ow}")
        nc.vector.tensor_copy(y_sb[:], ps[:])
        for oi in range(4):
            of = ow * 4 + oi
            nc.sync.dma_start(
                yT[of * 128:(of + 1) * 128, hf * 128:(hf + 1) * 128],
                y_sb[:, oi * 128:(oi + 1) * 128])


def make_mask_band() -> np.ndarray:
    p = np.arange(128)[:, None]
    j = np.arange(896)[None, :]
    return (p <= j - 384).astype(np.float32)


def prepare_in_maps(x, W_attn, W_proj):
    npdt = mybir.dt.np(DT)
    x = np.ascontiguousarray(np.asarray(x, dtype=np.float32))
    W_attn = np.ascontiguousarray(np.asarray(W_attn, dtype=np.float32))
    W_proj = np.ascontiguousarray(np.asarray(W_proj, dtype=np.float32))
    xT = np.ascontiguousarray(x.T).astype(npdt)
    wpT = np.ascontiguousarray(W_proj.T).astype(npdt)
    mask = make_mask_band()
    eye = np.eye(128, dtype=np.float32).astype(npdt)
    in_maps = []
    for r in range(N_CORES):
        rows = slice(r * FPC, (r + 1) * FPC)
        w_qkv = np.concatenate(
            [W_attn[0 * D:][rows], W_attn[1 * D:][rows], W_attn[2 * D:][rows]],
            axis=0)                                   # [768, D]
        in_maps.append({
            "xT": xT,
            "wqkvT": np.ascontiguousarray(w_qkv.T).astype(npdt),   # [D, 768]
            "wpT": wpT,                               # [D, D]
            "maskB": mask,
            "eyeB": eye,
        })
    return in_maps


def postprocess(results) -> np.ndarray:
    out = np.empty((T, D), dtype=np.float32)
    for i, r in enumerate(results):
        y = np.asarray(r["yT"]).astype(np.float32)    # [D, 2*128]
        out[128 * i:128 * (i + 1), :] = y[:, 0:128].T
        out[THALF + 128 * i:THALF + 128 * (i + 1), :] = y[:, 128:256].T
    return out


def kernel(x, W_attn, W_proj) -> np.ndarray:
    nc = build_nc()
    in_maps = prepare_in_maps(x, W_attn, W_proj)
    res = bass_utils.run_bass_kernel_spmd(
        nc, in_maps, core_ids=list(range(N_CORES)), trace=False)
    return postprocess(res.results)
